# revision 18
# baseline (speedup 1.0000x reference)
"""Trainium2 Bass kernel for MixtralBlockSparseTop2MLP grouped-GEMM MoE.

Problem: 4096 rows (sorted by expert), 8 experts, hidden=1024, ffn=3584.
  out[r] = silu(x[r] @ W1g[e(r)]) * (x[r] @ W1u[e(r)]) @ W2[e(r)]

Sharding: tensor-parallel over the ffn dimension. Each of the 8 cores gets
a 448-channel slice of every expert's gate/up/down weights and computes a
partial output for ALL 4096 rows; the host sums the 8 partials. All cores
run the identical program (segment structure baked from rows_for_experts at
call time), so one SPMD NEFF serves all 8 cores with per-core weight data.

v3 design notes (on top of the 184us v2):
  - the PE clock sits at 1.2GHz until ~25us into the kernel, then jumps to
    2.37GHz. Anything that idles the PE before that point is doubly wasted,
    so the startup path is restructured so real matmuls start at ~7.8us
    (the issue floor) with no DMA stalls: the first chunk's gemm1 runs
    K-MAJOR in two passes (m-slices u0..3, then u4..6), which only needs
    one x k-tile (1KB/part) + one w1 column-slice (1KB/part) to issue the
    first matmul. x and w1 arrive as fine per-k slices on the two HWDGE
    rings; each k-round at 1.2GHz (~1.7us) gives DMA time to stay ahead.
  - warmup matmuls cut 18 -> 4 (real work is the ramp fodder now).
  - exact-size xsb/osb tiles: per-chunk DMA is one contiguous line per
    partition (v2 sliced [*, :nch] out of 512-wide tiles, fragmenting each
    transfer into 4-8 descriptors per partition; the tail store drain was
    ~4.7us of tiny-packet backlog).
  - the last two chunks store every 2 m-tiles across sync/scalar/gpsimd
    rings (gpsimd's x loads are long done by then) so the final drain is
    short.
  - gemm2 is flipped: stationary = w2 [ffn_k x 128 H-cols], moving = the
    a-tiles [ffn_k x nch rows], psum = [128 H-cols x nch]. Cost scales with
    actual chunk rows; output is H-major, host transposes while summing.
  - the 4th gemm2 k-tile is zero-padded to 128 partitions (448 = 3.5
    k-tiles); zeros come from host-padded w2 and a persistent pair of
    zeroed a3 tiles. PE time per matmul is col-count-bound, so the padding
    costs no PE time, only ~1MB/core of extra w2 DMA (negligible).
  - chunks are balanced per segment (no chunk at the ~100ns issue floor);
    segments run largest-first (the big segment amortizes the startup
    weight stall and the 1.2GHz window), smallest last (short tail).

Compute dtype: bf16 matmul inputs with fp32 PSUM accumulation (fp32 matmul
is 4x slower; fp8 fails the 2e-2 gate - measured 3.8-6.5% rel err).
"""

import sys

sys.path.insert(0, "/opt/trn_rl_repo")

import numpy as np
import ml_dtypes

E, R, H, F = 8, 1024 * 4, 1024, 3584
FC = F // 8          # 448 ffn channels per core
P = 128
KO = H // P          # 8 k-tiles for gemm1
K2 = (FC + P - 1) // P   # 4 k-tiles for gemm2 (last has 64 rows)
KO2 = H // P         # 8 output m-tiles for gemm2
NCH = 512            # max row-chunk (PSUM bank free dim)

BF16 = ml_dtypes.bfloat16

# test.py introspection: last BassKernelResults from run_bass_kernel_spmd
LAST_RESULT = None

_PROGRAM_CACHE = {}


def _segments(rows_for_experts):
    """[(expert, row_start, n_rows)] for experts with n_rows > 0."""
    segs = []
    r0 = 0
    for e in range(E):
        n = int(rows_for_experts[e])
        if n > 0:
            segs.append((e, r0, n))
        r0 += n
    # largest segment first (amortizes the startup weight-load stall and
    # the low-clock window), smallest last (short end-of-kernel tail).
    segs.sort(key=lambda s: -s[2])
    return segs


def _chunk_sizes(n, greedy=False):
    """Split n rows into ceil(n/NCH) near-equal chunks (balanced so no
    chunk is tiny enough to be instruction-issue bound). greedy=True uses
    [512, 512, ..., rem] instead - used for the first segment so chunk 0
    is full-width."""
    k = (n + NCH - 1) // NCH
    if greedy:
        return [NCH] * (k - 1) + [n - NCH * (k - 1)]
    base, rem = divmod(n, k)
    return [base + 1] * rem + [base] * (k - rem)


def _chunk_list(segments):
    """[(expert, row_start, nch)] in program iteration order."""
    out = []
    for si, (e, r0, n_e) in enumerate(segments):
        c0 = 0
        for nch in _chunk_sizes(n_e, greedy=(si == 0)):
            out.append((e, r0 + c0, nch))
            c0 += nch
    return out


def _build_program(segments, act_mode="silu"):
    import concourse.mybir as mybir
    import concourse.tile as tile
    from concourse import bacc

    dt = mybir.dt
    nc = bacc.Bacc(None, target_bir_lowering=False, debug=False)

    chunks = _chunk_list(segments)
    n_chunks = len(chunks)
    # per-chunk packed x: chunk c occupies columns [KO*r : KO*(r+nch)] as a
    # row-major [KO, nch] block per partition (one contiguous line each).
    xTp = nc.declare_dram_parameter("xTp", [P, KO * R], dt.bfloat16, isOutput=False)
    w1 = nc.declare_dram_parameter("w1c", [E, P, KO, 2 * FC], dt.bfloat16, isOutput=False)
    w2 = nc.declare_dram_parameter("w2c", [E, P, K2, H], dt.bfloat16, isOutput=False)
    # per-chunk packed out: chunk c occupies columns [KO2*r : KO2*(r+nch)]
    # as a [KO2, nch] block per partition; out[r, m*128+p] = outp[p, ...].
    outp = nc.declare_dram_parameter("outp", [P, KO2 * R], dt.bfloat16, isOutput=True)

    silu = mybir.ActivationFunctionType.Silu
    sigmoid = mybir.ActivationFunctionType.Sigmoid
    copyf = mybir.ActivationFunctionType.Copy

    with tile.TileContext(nc) as tc:
        with (
            tc.tile_pool(name="w1p", bufs=4) as w1p,
            tc.tile_pool(name="w2p", bufs=3) as w2p,
            tc.tile_pool(name="xp", bufs=4) as xp,
            tc.tile_pool(name="apool", bufs=2) as apool,
            tc.tile_pool(name="a3pool", bufs=1) as a3pool,
            tc.tile_pool(name="spool", bufs=2) as spool,
            tc.tile_pool(name="opool", bufs=3) as opool,
            tc.tile_pool(name="hps", bufs=1, space="PSUM") as hps,
            tc.tile_pool(name="ops", bufs=3, space="PSUM") as ops,
        ):
            # a3 holds only channels 384:448 in rows 0:64; rows 64:128 must
            # be exact zeros (they multiply the zero-padded w2 k3 rows).
            # Persistent ping-pong pair, zeroed once.
            a3_tiles = [
                a3pool.tile([P, NCH], dt.bfloat16, tag=f"a3_{i}", name=f"a3_{i}")
                for i in range(2)
            ]
            for t3 in a3_tiles:
                nc.vector.memset(t3[:], 0.0)

            # PE warmup: dummy matmuls into a scratch psum bank nobody
            # reads. The clock governor runs the PE at 1.2GHz until it has
            # seen ~4-5.5us of continuous execution (and DOWN-shifts again
            # after a >=1us idle gap, costing several us of half-clock on
            # re-ramp). Real work can't start before ~11.5us anyway (two
            # rings x ~30pkt/us packet rate for the first x/w1 lines), so
            # burn the entire upshift latency on warmup: ~5.5us of 512-col
            # matmuls, ending ~12.5-13us just as the banked DMA data and
            # the 2.37GHz clock both become available.
            wsrc = a3pool.tile([P, NCH], dt.bfloat16, tag="wsrc", name="wsrc")
            nc.gpsimd.memset(wsrc[:], 0.0)
            wps = ops.tile([P, NCH], dt.float32, tag="o", name="warm")
            for _ in range(18):
                nc.tensor.matmul(wps[:, :NCH], wsrc[:, 0:P], wsrc[:, :NCH], start=True, stop=True)

            # gemm1 psum tags h0..h4 rotate via a global m-slice counter
            # (bufs=1 each -> 5 PSUM banks; reuse waits on act_mul reads).
            # 5 tags (not 4) so chunk 0's pass B (u4) starts on a fresh
            # bank instead of stalling ~1us on pass A's act_mul WAR.
            uc = [0]

            def h_tile(name):
                t = hps.tile([P, NCH], dt.float32, tag=f"h{uc[0] % 5}", name=name)
                uc[0] += 1
                return t

            chunk_idx = 0
            pending_gemm2 = None
            for (seg_idx, (e, r0, n_e)) in enumerate(segments):
                seg_sizes = _chunk_sizes(n_e, greedy=(seg_idx == 0))
                w1sb = w1p.tile([P, KO, 2 * FC], dt.bfloat16, tag="w1sb")
                # DMA throughput is packet-rate bound (one packet per
                # partition line), so fewer-but-bigger lines win; but a
                # transfer's semaphore fires only at the END, so data
                # needed progressively must be sliced.
                if seg_idx == 0:
                    # Startup supply: every transfer costs ~4.3-5us on its
                    # queue (128 per-partition lines at ~25-30 lines/us,
                    # roughly independent of line size); sync flows from
                    # ~8us, scalar from ~10.5us, gpsimd (SWDGE) from
                    # ~10.8us. Spread chunk 0's working set across all
                    # THREE queues in k-consumption order so the full set
                    # is banked by ~19.5us: sync carries x k0:4, scalar
                    # carries w1 k0:3 + k3:6, gpsimd carries x k4:8 +
                    # w1 k6:8 + w2[e]. The early k-major rounds run
                    # DMA-paced at the 1.2GHz startup clock.
                    # Queue delivery slots (measured): sync ~12.3/16.9/21.3,
                    # scalar ~14.3/18.8, gpsimd ~17.9/22.3us. Matched to
                    # the k-round need times at full clock (k0 @14.4, k3
                    # @17.0, k4 @17.9, k6 @19.6); chunk 1's x rides sync
                    # slot 3 (the gpsimd queue would deliver it too late
                    # behind w2).
                    n0 = seg_sizes[0]
                    xsb0 = xp.tile([P, KO, n0], dt.bfloat16, tag="xsb", name="xsb0")
                    off = KO * r0
                    nc.sync.dma_start(
                        xsb0[:, 0:4, :], xTp[:, off : off + 4 * n0]
                    )
                    nc.scalar.dma_start(w1sb[:, 0:3, :], w1[e, :, 0:3, :])
                    nc.gpsimd.dma_start(
                        xsb0[:, 4:8, :], xTp[:, off + 4 * n0 : off + 8 * n0]
                    )
                    nc.sync.dma_start(w1sb[:, 3:6, :], w1[e, :, 3:6, :])
                    nc.scalar.dma_start(w1sb[:, 6:8, :], w1[e, :, 6:8, :])
                else:
                    # two half-expert transfers (7KB lines); the second
                    # expert's ride both rings (startup window is BW-bound)
                    xsb0 = None
                    KH = KO // 2
                    nc.sync.dma_start(w1sb[:, :KH, :], w1[e, :, :KH, :])
                    eng2 = nc.scalar if seg_idx == 1 else nc.sync
                    eng2.dma_start(w1sb[:, KH:, :], w1[e, :, KH:, :])
                w2sb = w2p.tile([P, K2, H], dt.bfloat16, tag="w2sb")
                if seg_idx == 0:
                    # on gpsimd behind chunk 0's x/w1 pieces: lands ~24us,
                    # needed by gemm2[c0] at ~26us. (Deferred to scalar it
                    # arrives after gemm1[c0] ends and stalls the PE.)
                    nc.gpsimd.dma_start(w2sb[:], w2[e])
                    w2_pending = None
                elif seg_idx == 1:
                    # second expert: keep the 1MB w2 load out of the
                    # startup window (it is not needed until this expert's
                    # first gemm2, one chunk later); emit it after the
                    # first chunk's gemm1 instructions instead.
                    w2_pending = (w2sb, e)
                else:
                    nc.scalar.dma_start(w2sb[:], w2[e])
                    w2_pending = None

                c0 = 0
                for ci_seg, nch in enumerate(seg_sizes):
                    r = r0 + c0
                    c0 += nch

                    if xsb0 is not None:
                        xsb, xsb0 = xsb0, None
                    else:
                        xsb = xp.tile([P, KO, nch], dt.bfloat16, tag="xsb", name="xsb")
                        off = KO * r
                        # chunk 1's x on sync (slot 3, ~21.5us): the
                        # gpsimd queue has w1 k3:6 + w2[e0'] ahead of it
                        # and would starve gemm1[c1] at ~30us.
                        xeng = nc.sync if chunk_idx == 1 else nc.gpsimd
                        xeng.dma_start(
                            xsb[:], xTp[:, off : off + KO * nch]
                        )

                    # gemm1: 7 packed m-slices [gate_u(64) | up_u(64)];
                    # psum_u partitions 0:64 = gate, 64:128 = up.
                    # silu via ACT into a 64-row tmp, then DVE cross-base
                    # multiply into the packed a k-tiles. a3 rows 64:128
                    # are never written or read (448 = 3.5 k-tiles).
                    a_tiles = [
                        apool.tile([P, NCH], dt.bfloat16, tag=f"a{j}", name=f"a{j}")
                        if j < 3
                        else a3_tiles[chunk_idx % 2]
                        for j in range(K2)
                    ]

                    def act_mul(u, hu_ps):
                        stmp = spool.tile([64, NCH], dt.bfloat16, tag="stmp", name="stmp")
                        if act_mode == "silu":
                            nc.scalar.activation(
                                stmp[:, :nch], hu_ps[0:64, :nch], silu
                            )
                        else:  # silu(g) = g * sigmoid(g); CoreSim lacks Silu
                            nc.scalar.activation(
                                stmp[:, :nch], hu_ps[0:64, :nch], sigmoid
                            )
                            nc.vector.tensor_mul(
                                stmp[:, :nch], stmp[:, :nch], hu_ps[0:64, :nch]
                            )
                        lo = 64 * (u % 2)
                        nc.vector.tensor_mul(
                            a_tiles[u // 2][lo : lo + 64, :nch],
                            stmp[:, :nch],
                            hu_ps[64:128, :nch],
                        )

                    if chunk_idx == 0:
                        # k-major two-pass gemm1: pass A (u0..3) only needs
                        # the k-th x slice + w1 column-slice per round, so
                        # the PE starts ~5us earlier than m-major would.
                        for ulist in (range(0, 4), range(4, 7)):
                            hu_list = [h_tile(f"h{u}") for u in ulist]
                            for k in range(KO):
                                for hu_ps, u in zip(hu_list, ulist):
                                    nc.tensor.matmul(
                                        hu_ps[:, :nch],
                                        w1sb[:, k, P * u : P * u + P],
                                        xsb[:, k, :nch],
                                        start=(k == 0),
                                        stop=(k == KO - 1),
                                    )
                            for hu_ps, u in zip(hu_list, ulist):
                                act_mul(u, hu_ps)
                    else:
                        for u in range(7):
                            hu_ps = h_tile(f"h{u}")
                            for k in range(KO):
                                nc.tensor.matmul(
                                    hu_ps[:, :nch],
                                    w1sb[:, k, P * u : P * u + P],
                                    xsb[:, k, :nch],
                                    start=(k == 0),
                                    stop=(k == KO - 1),
                                )
                            act_mul(u, hu_ps)
                    if w2_pending is not None:
                        nc.scalar.dma_start(w2_pending[0][:], w2[w2_pending[1]])
                        w2_pending = None

                    # gemm2 (emitted one chunk behind gemm1 so the PE never
                    # waits on this chunk's silu/mul chain). Flipped layout:
                    # for each 128-col H tile m: psum[128, nch] accumulates
                    # over 4 ffn k-tiles (last only 64 partitions), cast to
                    # osb[:, m, :], then store packed slices. The last two
                    # chunks store every 2 m-tiles across three rings so
                    # the final drain is short (gpsimd is idle by then).
                    is_tail = chunk_idx >= n_chunks - 2
                    is_last = chunk_idx == n_chunks - 1

                    def gemm2(nch=nch, r=r, a_tiles=a_tiles, w2sb=w2sb,
                              store_every=(2 if is_tail else 4),
                              last=is_last):
                        osb = opool.tile(
                            [P, KO2, nch], dt.bfloat16, tag="osb", name="osb"
                        )
                        rings = (
                            [nc.sync, nc.scalar, nc.gpsimd, nc.scalar]
                            if last
                            else [nc.sync, nc.scalar, nc.sync, nc.scalar]
                        )
                        for m in range(KO2):
                            o_ps = ops.tile([P, NCH], dt.float32, tag="o", name=f"o{m}")
                            for k in range(K2):
                                nc.tensor.matmul(
                                    o_ps[:, :nch],
                                    w2sb[:, k, P * m : P * m + P],
                                    a_tiles[k][:, :nch],
                                    start=(k == 0),
                                    stop=(k == K2 - 1),
                                )
                            if m % 2 == 0:
                                nc.vector.tensor_copy(
                                    osb[:, m, :], o_ps[:, :nch]
                                )
                            else:
                                nc.scalar.activation(
                                    osb[:, m, :], o_ps[:, :nch], copyf
                                )
                            if (m + 1) % store_every == 0:
                                lo = m + 1 - store_every
                                off = KO2 * r + lo * nch
                                eng = rings[(lo // store_every) % len(rings)]
                                eng.dma_start(
                                    outp[:, off : off + store_every * nch],
                                    osb[:, lo : m + 1, :],
                                )

                    if pending_gemm2 is not None:
                        pending_gemm2()
                    pending_gemm2 = gemm2
                    chunk_idx += 1
            pending_gemm2()

    nc.compile()
    return nc


def _prepare_inputs(hidden_states, w1, w2, chunks):
    """Host-side shard/layout/cast. Returns (xTp, [w1c], [w2c])."""
    x = np.asarray(hidden_states, dtype=np.float32)
    w1 = np.asarray(w1, dtype=np.float32)
    w2 = np.asarray(w2, dtype=np.float32)

    xb = x.astype(BF16)          # [R, H]
    w1b = w1.astype(BF16)        # [E, H, 2F]
    w2b = w2.astype(BF16)        # [E, F, H]

    # xTflat[p, k, r] = x[r, 128*k + p]
    xTflat = np.ascontiguousarray(xb.T.reshape(KO, P, R).transpose(1, 0, 2))
    # per-chunk packed: chunk at row r, size nch -> cols [KO*r : KO*(r+nch)]
    xTp = np.empty((P, KO * R), dtype=BF16)
    for (_, r, nch) in chunks:
        xTp[:, KO * r : KO * (r + nch)] = xTflat[:, :, r : r + nch].reshape(
            P, KO * nch
        )

    w1cs, w2cs = [], []
    for c in range(8):
        gate = w1b[:, :, c * FC : (c + 1) * FC]
        up = w1b[:, :, F + c * FC : F + (c + 1) * FC]
        # interleave 64-channel blocks: [G0|U0|G1|U1|...|G6|U6] so each
        # 128-column m-slice u packs gate_u in psum partitions 0:64 and
        # up_u in 64:128.
        w1cat = np.ascontiguousarray(
            np.stack(
                [gate.reshape(E, H, FC // 64, 64), up.reshape(E, H, FC // 64, 64)],
                axis=3,
            ).reshape(E, H, 2 * FC)
        )
        w1c = np.ascontiguousarray(
            w1cat.reshape(E, KO, P, 2 * FC).transpose(0, 2, 1, 3)
        )
        w2pad = np.zeros((E, K2 * P, H), dtype=BF16)
        w2pad[:, :FC, :] = w2b[:, c * FC : (c + 1) * FC, :]
        w2c = np.ascontiguousarray(
            w2pad.reshape(E, K2, P, H).transpose(0, 2, 1, 3)
        )                                               # [E, P, K2, H]
        w1cs.append(w1c)
        w2cs.append(w2c)
    return xTp, w1cs, w2cs


def kernel(hidden_states, w1, w2, rows_for_experts):
    global LAST_RESULT
    from concourse.bass_utils import run_bass_kernel_spmd

    segs = _segments(np.asarray(rows_for_experts))
    if not segs:
        return np.zeros((R, H), dtype=np.float32)
    key = tuple(segs)
    nc = _PROGRAM_CACHE.get(key)
    if nc is None:
        nc = _build_program(segs)
        _PROGRAM_CACHE[key] = nc

    chunks = _chunk_list(segs)
    xTp, w1cs, w2cs = _prepare_inputs(hidden_states, w1, w2, chunks)
    in_maps = [
        {"xTp": xTp, "w1c": w1cs[c], "w2c": w2cs[c]} for c in range(8)
    ]
    res = run_bass_kernel_spmd(nc, in_maps, core_ids=list(range(8)))
    LAST_RESULT = res

    acc = np.zeros((R, H), dtype=np.float32)
    for c in range(8):
        flat = res.results[c]["outp"]  # [P, KO2*R] bf16, per-chunk packed
        for (_, r, nch) in chunks:
            blk = flat[:, KO2 * r : KO2 * (r + nch)].reshape(P, KO2, nch)
            # out[r+j, m*128+p] = blk[p, m, j]
            acc[r : r + nch] += (
                blk.transpose(1, 0, 2).reshape(H, nch).T.astype(np.float32)
            )
    return acc


# revision 19
# speedup vs baseline: 1.0124x; 1.0124x over previous
"""Trainium2 Bass kernel for MixtralBlockSparseTop2MLP grouped-GEMM MoE.

Problem: 4096 rows (sorted by expert), 8 experts, hidden=1024, ffn=3584.
  out[r] = silu(x[r] @ W1g[e(r)]) * (x[r] @ W1u[e(r)]) @ W2[e(r)]

Sharding: tensor-parallel over the ffn dimension. Each of the 8 cores gets
a 448-channel slice of every expert's gate/up/down weights and computes a
partial output for ALL 4096 rows; the host sums the 8 partials. All cores
run the identical program (segment structure baked from rows_for_experts at
call time), so one SPMD NEFF serves all 8 cores with per-core weight data.

v3 design notes (on top of the 184us v2):
  - the PE clock sits at 1.2GHz until ~25us into the kernel, then jumps to
    2.37GHz. Anything that idles the PE before that point is doubly wasted,
    so the startup path is restructured so real matmuls start at ~7.8us
    (the issue floor) with no DMA stalls: the first chunk's gemm1 runs
    K-MAJOR in two passes (m-slices u0..3, then u4..6), which only needs
    one x k-tile (1KB/part) + one w1 column-slice (1KB/part) to issue the
    first matmul. x and w1 arrive as fine per-k slices on the two HWDGE
    rings; each k-round at 1.2GHz (~1.7us) gives DMA time to stay ahead.
  - warmup matmuls cut 18 -> 4 (real work is the ramp fodder now).
  - exact-size xsb/osb tiles: per-chunk DMA is one contiguous line per
    partition (v2 sliced [*, :nch] out of 512-wide tiles, fragmenting each
    transfer into 4-8 descriptors per partition; the tail store drain was
    ~4.7us of tiny-packet backlog).
  - the last two chunks store every 2 m-tiles across sync/scalar/gpsimd
    rings (gpsimd's x loads are long done by then) so the final drain is
    short.
  - gemm2 is flipped: stationary = w2 [ffn_k x 128 H-cols], moving = the
    a-tiles [ffn_k x nch rows], psum = [128 H-cols x nch]. Cost scales with
    actual chunk rows; output is H-major, host transposes while summing.
  - the 4th gemm2 k-tile is zero-padded to 128 partitions (448 = 3.5
    k-tiles); zeros come from host-padded w2 and a persistent pair of
    zeroed a3 tiles. PE time per matmul is col-count-bound, so the padding
    costs no PE time, only ~1MB/core of extra w2 DMA (negligible).
  - chunks are balanced per segment (no chunk at the ~100ns issue floor);
    segments run largest-first (the big segment amortizes the startup
    weight stall and the 1.2GHz window), smallest last (short tail).

Compute dtype: bf16 matmul inputs with fp32 PSUM accumulation (fp32 matmul
is 4x slower; fp8 fails the 2e-2 gate - measured 3.8-6.5% rel err).
"""

import sys

sys.path.insert(0, "/opt/trn_rl_repo")

import numpy as np
import ml_dtypes

E, R, H, F = 8, 1024 * 4, 1024, 3584
FC = F // 8          # 448 ffn channels per core
P = 128
KO = H // P          # 8 k-tiles for gemm1
K2 = (FC + P - 1) // P   # 4 k-tiles for gemm2 (last has 64 rows)
KO2 = H // P         # 8 output m-tiles for gemm2
NCH = 512            # max row-chunk (PSUM bank free dim)

BF16 = ml_dtypes.bfloat16

# test.py introspection: last BassKernelResults from run_bass_kernel_spmd
LAST_RESULT = None

_PROGRAM_CACHE = {}


def _segments(rows_for_experts):
    """[(expert, row_start, n_rows)] for experts with n_rows > 0."""
    segs = []
    r0 = 0
    for e in range(E):
        n = int(rows_for_experts[e])
        if n > 0:
            segs.append((e, r0, n))
        r0 += n
    # largest segment first (amortizes the startup weight-load stall and
    # the low-clock window), smallest last (short end-of-kernel tail).
    segs.sort(key=lambda s: -s[2])
    return segs


def _chunk_sizes(n, greedy=False):
    """Split n rows into ceil(n/NCH) near-equal chunks (balanced so no
    chunk is tiny enough to be instruction-issue bound). greedy=True uses
    [512, 512, ..., rem] instead - used for the first segment so chunk 0
    is full-width."""
    k = (n + NCH - 1) // NCH
    if greedy:
        return [NCH] * (k - 1) + [n - NCH * (k - 1)]
    base, rem = divmod(n, k)
    return [base + 1] * rem + [base] * (k - rem)


def _chunk_list(segments):
    """[(expert, row_start, nch)] in program iteration order."""
    out = []
    for si, (e, r0, n_e) in enumerate(segments):
        c0 = 0
        for nch in _chunk_sizes(n_e, greedy=(si == 0)):
            out.append((e, r0 + c0, nch))
            c0 += nch
    return out


def _build_program(segments, act_mode="silu"):
    import concourse.mybir as mybir
    import concourse.tile as tile
    from concourse import bacc

    dt = mybir.dt
    nc = bacc.Bacc(None, target_bir_lowering=False, debug=False)

    chunks = _chunk_list(segments)
    n_chunks = len(chunks)
    # per-chunk packed x: chunk c occupies columns [KO*r : KO*(r+nch)] as a
    # row-major [KO, nch] block per partition (one contiguous line each).
    xTp = nc.declare_dram_parameter("xTp", [P, KO * R], dt.bfloat16, isOutput=False)
    w1 = nc.declare_dram_parameter("w1c", [E, P, KO, 2 * FC], dt.bfloat16, isOutput=False)
    w2 = nc.declare_dram_parameter("w2c", [E, P, K2, H], dt.bfloat16, isOutput=False)
    # per-chunk packed out: chunk c occupies columns [KO2*r : KO2*(r+nch)]
    # as a [KO2, nch] block per partition; out[r, m*128+p] = outp[p, ...].
    outp = nc.declare_dram_parameter("outp", [P, KO2 * R], dt.bfloat16, isOutput=True)

    silu = mybir.ActivationFunctionType.Silu
    sigmoid = mybir.ActivationFunctionType.Sigmoid
    copyf = mybir.ActivationFunctionType.Copy

    with tile.TileContext(nc) as tc:
        with (
            tc.tile_pool(name="w1p", bufs=4) as w1p,
            tc.tile_pool(name="w2p", bufs=3) as w2p,
            tc.tile_pool(name="xp", bufs=4) as xp,
            tc.tile_pool(name="apool", bufs=2) as apool,
            tc.tile_pool(name="a3pool", bufs=1) as a3pool,
            tc.tile_pool(name="spool", bufs=2) as spool,
            tc.tile_pool(name="opool", bufs=3) as opool,
            tc.tile_pool(name="hps", bufs=1, space="PSUM") as hps,
            tc.tile_pool(name="ops", bufs=3, space="PSUM") as ops,
        ):
            # a3 holds only channels 384:448 in rows 0:64; rows 64:128 must
            # be exact zeros (they multiply the zero-padded w2 k3 rows).
            # Persistent ping-pong pair, zeroed once.
            a3_tiles = [
                a3pool.tile([P, NCH], dt.bfloat16, tag=f"a3_{i}", name=f"a3_{i}")
                for i in range(2)
            ]
            for t3 in a3_tiles:
                nc.vector.memset(t3[:], 0.0)

            # PE warmup: dummy matmuls into a scratch psum bank nobody
            # reads. The clock governor runs the PE at 1.2GHz until it has
            # seen ~4-5.5us of continuous execution (and DOWN-shifts again
            # after a >=1us idle gap, costing several us of half-clock on
            # re-ramp). Real work can't start before ~11.5us anyway (two
            # rings x ~30pkt/us packet rate for the first x/w1 lines), so
            # burn the entire upshift latency on warmup: ~5.5us of 512-col
            # matmuls, ending ~12.5-13us just as the banked DMA data and
            # the 2.37GHz clock both become available.
            wsrc = a3pool.tile([P, NCH], dt.bfloat16, tag="wsrc", name="wsrc")
            nc.gpsimd.memset(wsrc[:], 0.0)
            wps = ops.tile([P, NCH], dt.float32, tag="o", name="warm")
            for _ in range(18):
                nc.tensor.matmul(wps[:, :NCH], wsrc[:, 0:P], wsrc[:, :NCH], start=True, stop=True)

            # gemm1 psum tags h0..h4 rotate via a global m-slice counter
            # (bufs=1 each -> 5 PSUM banks; reuse waits on act_mul reads).
            # 5 tags (not 4) so chunk 0's pass B (u4) starts on a fresh
            # bank instead of stalling ~1us on pass A's act_mul WAR.
            uc = [0]

            def h_tile(name):
                t = hps.tile([P, NCH], dt.float32, tag=f"h{uc[0] % 5}", name=name)
                uc[0] += 1
                return t

            chunk_idx = 0
            pending_gemm2 = None
            for (seg_idx, (e, r0, n_e)) in enumerate(segments):
                seg_sizes = _chunk_sizes(n_e, greedy=(seg_idx == 0))
                w1sb = w1p.tile([P, KO, 2 * FC], dt.bfloat16, tag="w1sb")
                # DMA throughput is packet-rate bound (one packet per
                # partition line), so fewer-but-bigger lines win; but a
                # transfer's semaphore fires only at the END, so data
                # needed progressively must be sliced.
                if seg_idx == 0:
                    # Startup supply: every transfer costs ~4.3-5us on its
                    # queue (128 per-partition lines at ~25-30 lines/us,
                    # roughly independent of line size); sync flows from
                    # ~8us, scalar from ~10.5us, gpsimd (SWDGE) from
                    # ~10.8us. Spread chunk 0's working set across all
                    # THREE queues in k-consumption order so the full set
                    # is banked by ~19.5us: sync carries x k0:4, scalar
                    # carries w1 k0:3 + k3:6, gpsimd carries x k4:8 +
                    # w1 k6:8 + w2[e]. The early k-major rounds run
                    # DMA-paced at the 1.2GHz startup clock.
                    # Queue delivery slots (measured): sync ~12.3/17/21.5,
                    # scalar ~14.8/19.5, gpsimd ~17.9/22.3us. Matched to
                    # k-round need times; chunk 1's x rides sync slot 3
                    # (the gpsimd queue would deliver it too late behind
                    # w2).
                    n0 = seg_sizes[0]
                    xsb0 = xp.tile([P, KO, n0], dt.bfloat16, tag="xsb", name="xsb0")
                    off = KO * r0
                    nc.sync.dma_start(
                        xsb0[:, 0:4, :], xTp[:, off : off + 4 * n0]
                    )
                    nc.scalar.dma_start(w1sb[:, 0:3, :], w1[e, :, 0:3, :])
                    nc.gpsimd.dma_start(w1sb[:, 3:6, :], w1[e, :, 3:6, :])
                    nc.sync.dma_start(
                        xsb0[:, 4:8, :], xTp[:, off + 4 * n0 : off + 8 * n0]
                    )
                    nc.scalar.dma_start(w1sb[:, 6:8, :], w1[e, :, 6:8, :])
                else:
                    # two half-expert transfers (7KB lines); the second
                    # expert's ride both rings (startup window is BW-bound)
                    xsb0 = None
                    KH = KO // 2
                    nc.sync.dma_start(w1sb[:, :KH, :], w1[e, :, :KH, :])
                    eng2 = nc.scalar if seg_idx == 1 else nc.sync
                    eng2.dma_start(w1sb[:, KH:, :], w1[e, :, KH:, :])
                w2sb = w2p.tile([P, K2, H], dt.bfloat16, tag="w2sb")
                if seg_idx == 0:
                    # on gpsimd behind chunk 0's x/w1 pieces: lands ~24us,
                    # needed by gemm2[c0] at ~26us. (Deferred to scalar it
                    # arrives after gemm1[c0] ends and stalls the PE.)
                    nc.gpsimd.dma_start(w2sb[:], w2[e])
                    w2_pending = None
                elif seg_idx == 1:
                    # second expert: keep the 1MB w2 load out of the
                    # startup window (it is not needed until this expert's
                    # first gemm2, one chunk later); emit it after the
                    # first chunk's gemm1 instructions instead.
                    w2_pending = (w2sb, e)
                else:
                    nc.scalar.dma_start(w2sb[:], w2[e])
                    w2_pending = None

                c0 = 0
                for ci_seg, nch in enumerate(seg_sizes):
                    r = r0 + c0
                    c0 += nch

                    if xsb0 is not None:
                        xsb, xsb0 = xsb0, None
                    else:
                        xsb = xp.tile([P, KO, nch], dt.bfloat16, tag="xsb", name="xsb")
                        off = KO * r
                        # chunk 1's x on sync (slot 3, ~21.5us): the
                        # gpsimd queue has w1 k3:6 + w2[e0'] ahead of it
                        # and would starve gemm1[c1] at ~30us.
                        xeng = nc.sync if chunk_idx == 1 else nc.gpsimd
                        xeng.dma_start(
                            xsb[:], xTp[:, off : off + KO * nch]
                        )

                    # gemm1: 7 packed m-slices [gate_u(64) | up_u(64)];
                    # psum_u partitions 0:64 = gate, 64:128 = up.
                    # silu via ACT into a 64-row tmp, then DVE cross-base
                    # multiply into the packed a k-tiles. a3 rows 64:128
                    # are never written or read (448 = 3.5 k-tiles).
                    a_tiles = [
                        apool.tile([P, NCH], dt.bfloat16, tag=f"a{j}", name=f"a{j}")
                        if j < 3
                        else a3_tiles[chunk_idx % 2]
                        for j in range(K2)
                    ]

                    def act_mul(u, hu_ps):
                        stmp = spool.tile([64, NCH], dt.bfloat16, tag="stmp", name="stmp")
                        if act_mode == "silu":
                            nc.scalar.activation(
                                stmp[:, :nch], hu_ps[0:64, :nch], silu
                            )
                        else:  # silu(g) = g * sigmoid(g); CoreSim lacks Silu
                            nc.scalar.activation(
                                stmp[:, :nch], hu_ps[0:64, :nch], sigmoid
                            )
                            nc.vector.tensor_mul(
                                stmp[:, :nch], stmp[:, :nch], hu_ps[0:64, :nch]
                            )
                        lo = 64 * (u % 2)
                        nc.vector.tensor_mul(
                            a_tiles[u // 2][lo : lo + 64, :nch],
                            stmp[:, :nch],
                            hu_ps[64:128, :nch],
                        )

                    if chunk_idx == 0:
                        # k-major two-pass gemm1: pass A (u0..3) only needs
                        # the k-th x slice + w1 column-slice per round, so
                        # the PE starts ~5us earlier than m-major would.
                        for ulist in (range(0, 4), range(4, 7)):
                            hu_list = [h_tile(f"h{u}") for u in ulist]
                            for k in range(KO):
                                for hu_ps, u in zip(hu_list, ulist):
                                    nc.tensor.matmul(
                                        hu_ps[:, :nch],
                                        w1sb[:, k, P * u : P * u + P],
                                        xsb[:, k, :nch],
                                        start=(k == 0),
                                        stop=(k == KO - 1),
                                    )
                            for hu_ps, u in zip(hu_list, ulist):
                                act_mul(u, hu_ps)
                    else:
                        for u in range(7):
                            hu_ps = h_tile(f"h{u}")
                            for k in range(KO):
                                nc.tensor.matmul(
                                    hu_ps[:, :nch],
                                    w1sb[:, k, P * u : P * u + P],
                                    xsb[:, k, :nch],
                                    start=(k == 0),
                                    stop=(k == KO - 1),
                                )
                            act_mul(u, hu_ps)
                    if w2_pending is not None:
                        nc.scalar.dma_start(w2_pending[0][:], w2[w2_pending[1]])
                        w2_pending = None

                    # gemm2 (emitted one chunk behind gemm1 so the PE never
                    # waits on this chunk's silu/mul chain). Flipped layout:
                    # for each 128-col H tile m: psum[128, nch] accumulates
                    # over 4 ffn k-tiles (last only 64 partitions), cast to
                    # osb[:, m, :], then store packed slices. The last two
                    # chunks store every 2 m-tiles across three rings so
                    # the final drain is short (gpsimd is idle by then).
                    is_tail = chunk_idx >= n_chunks - 2
                    is_last = chunk_idx == n_chunks - 1

                    def gemm2(nch=nch, r=r, a_tiles=a_tiles, w2sb=w2sb,
                              store_every=(2 if is_tail else 4),
                              last=is_last):
                        osb = opool.tile(
                            [P, KO2, nch], dt.bfloat16, tag="osb", name="osb"
                        )
                        rings = (
                            [nc.sync, nc.scalar, nc.gpsimd, nc.scalar]
                            if last
                            else [nc.sync, nc.scalar, nc.sync, nc.scalar]
                        )
                        for m in range(KO2):
                            o_ps = ops.tile([P, NCH], dt.float32, tag="o", name=f"o{m}")
                            for k in range(K2):
                                nc.tensor.matmul(
                                    o_ps[:, :nch],
                                    w2sb[:, k, P * m : P * m + P],
                                    a_tiles[k][:, :nch],
                                    start=(k == 0),
                                    stop=(k == K2 - 1),
                                )
                            if m % 2 == 0:
                                nc.vector.tensor_copy(
                                    osb[:, m, :], o_ps[:, :nch]
                                )
                            else:
                                nc.scalar.activation(
                                    osb[:, m, :], o_ps[:, :nch], copyf
                                )
                            if (m + 1) % store_every == 0:
                                lo = m + 1 - store_every
                                off = KO2 * r + lo * nch
                                eng = rings[(lo // store_every) % len(rings)]
                                eng.dma_start(
                                    outp[:, off : off + store_every * nch],
                                    osb[:, lo : m + 1, :],
                                )

                    if pending_gemm2 is not None:
                        pending_gemm2()
                    pending_gemm2 = gemm2
                    chunk_idx += 1
            pending_gemm2()

    nc.compile()
    return nc


def _prepare_inputs(hidden_states, w1, w2, chunks):
    """Host-side shard/layout/cast. Returns (xTp, [w1c], [w2c])."""
    x = np.asarray(hidden_states, dtype=np.float32)
    w1 = np.asarray(w1, dtype=np.float32)
    w2 = np.asarray(w2, dtype=np.float32)

    xb = x.astype(BF16)          # [R, H]
    w1b = w1.astype(BF16)        # [E, H, 2F]
    w2b = w2.astype(BF16)        # [E, F, H]

    # xTflat[p, k, r] = x[r, 128*k + p]
    xTflat = np.ascontiguousarray(xb.T.reshape(KO, P, R).transpose(1, 0, 2))
    # per-chunk packed: chunk at row r, size nch -> cols [KO*r : KO*(r+nch)]
    xTp = np.empty((P, KO * R), dtype=BF16)
    for (_, r, nch) in chunks:
        xTp[:, KO * r : KO * (r + nch)] = xTflat[:, :, r : r + nch].reshape(
            P, KO * nch
        )

    w1cs, w2cs = [], []
    for c in range(8):
        gate = w1b[:, :, c * FC : (c + 1) * FC]
        up = w1b[:, :, F + c * FC : F + (c + 1) * FC]
        # interleave 64-channel blocks: [G0|U0|G1|U1|...|G6|U6] so each
        # 128-column m-slice u packs gate_u in psum partitions 0:64 and
        # up_u in 64:128.
        w1cat = np.ascontiguousarray(
            np.stack(
                [gate.reshape(E, H, FC // 64, 64), up.reshape(E, H, FC // 64, 64)],
                axis=3,
            ).reshape(E, H, 2 * FC)
        )
        w1c = np.ascontiguousarray(
            w1cat.reshape(E, KO, P, 2 * FC).transpose(0, 2, 1, 3)
        )
        w2pad = np.zeros((E, K2 * P, H), dtype=BF16)
        w2pad[:, :FC, :] = w2b[:, c * FC : (c + 1) * FC, :]
        w2c = np.ascontiguousarray(
            w2pad.reshape(E, K2, P, H).transpose(0, 2, 1, 3)
        )                                               # [E, P, K2, H]
        w1cs.append(w1c)
        w2cs.append(w2c)
    return xTp, w1cs, w2cs


def kernel(hidden_states, w1, w2, rows_for_experts):
    global LAST_RESULT
    from concourse.bass_utils import run_bass_kernel_spmd

    segs = _segments(np.asarray(rows_for_experts))
    if not segs:
        return np.zeros((R, H), dtype=np.float32)
    key = tuple(segs)
    nc = _PROGRAM_CACHE.get(key)
    if nc is None:
        nc = _build_program(segs)
        _PROGRAM_CACHE[key] = nc

    chunks = _chunk_list(segs)
    xTp, w1cs, w2cs = _prepare_inputs(hidden_states, w1, w2, chunks)
    in_maps = [
        {"xTp": xTp, "w1c": w1cs[c], "w2c": w2cs[c]} for c in range(8)
    ]
    res = run_bass_kernel_spmd(nc, in_maps, core_ids=list(range(8)))
    LAST_RESULT = res

    acc = np.zeros((R, H), dtype=np.float32)
    for c in range(8):
        flat = res.results[c]["outp"]  # [P, KO2*R] bf16, per-chunk packed
        for (_, r, nch) in chunks:
            blk = flat[:, KO2 * r : KO2 * (r + nch)].reshape(P, KO2, nch)
            # out[r+j, m*128+p] = blk[p, m, j]
            acc[r : r + nch] += (
                blk.transpose(1, 0, 2).reshape(H, nch).T.astype(np.float32)
            )
    return acc
